# revision 45
# baseline (speedup 1.0000x reference)
"""Trainium2 Bass kernel for nn_ContinuousThoughtBlock (sharded thought MLP).

Strategy: batch-parallel for h/gate/output (core r owns batch r), but the
thought-phase MLP is model-parallel over H across all 8 cores:

  context_b = mean_L(h_b) and thought0_b are computed on core b, then one
  small AllGather shares [ctx || th0] for all batches; every core then
  evolves ALL 64 (path, batch) columns through the 4 residual-MLP steps,
  contracting only its own H-slice (W1[:, r*512:], W2[r*512:, :]); the
  partial dense2 outputs are AllReduced (128KB bf16) per step.  PE
  efficiency per matmul rises from 8 to 64 streamed columns and per-core
  weight DMA drops 16MB -> 2MB.

  amps/collapse are computed replicated for all 8 batches; a one-hot
  `bsel` input row selects the core's own final thought for bc (SPMD
  graphs must be identical across cores, so per-core choices ride in via
  inputs).

  gate = sigmoid(h @ Wg + bg) stays resident in SBUF (no DRAM spill);
  out = LN_D(h + gate * bc) written as bf16 (host converts to f32).

  A dummy AllGather fires at t=0 so the one-time collective-init /
  core-start skew is absorbed under the loads instead of the first real
  collective.

Layouts: all big tensors host-repacked partition-major (one contiguous
descriptor per partition row); h additionally comes pre-transposed (hT)
so no on-chip SBUF<->SBUF transposes are needed (those ride the gpsimd
SWDGE ring and serialize behind collectives).

Queues: sync = Wg, Wagg, AG/AR bounce writes, h, Wbc, odd out tiles;
scalar = hT, W1s/W2s, even out tiles; gpsimd = collectives + collective
result reads (+ tail elementwise help).
"""

import numpy as np

import concourse.bass as bass
import concourse.mybir as mybir
import concourse.tile as tile
from concourse import bacc
from concourse.bass_utils import run_bass_kernel_spmd
from concourse.masks import make_identity

# Problem constants (hardcoded per harness contract).
B, L, D, H = 8, 2048, 1024, 4096
NUM_PATHS = 8
NUM_STEPS = 4
PRUNE = 0.1
EPS = 1e-6
KD = D // 128     # 8  D-chunks
ML = L // 128     # 16 L-tiles
HSL = H // B      # 512 per-core H slice
KHS = HSL // 128  # 4  H-slice chunks
NC = 64           # thought columns = B batches x NUM_PATHS paths, c = b*8+p
INV_SQRT_D = 1.0 / float(np.sqrt(np.float32(D)))

F32 = mybir.dt.float32
BF16 = mybir.dt.bfloat16
AF = mybir.ActivationFunctionType
ALU = mybir.AluOpType
AX = mybir.AxisListType
RG = [list(range(B))]

WEIGHT_NAMES = [
    "input_norm_gamma", "input_norm_beta",
    "aggregator_weight", "aggregator_bias",
    "projector_norm_gamma", "projector_norm_beta",
    "projector_dense1_weight", "projector_dense1_bias",
    "projector_dense2_weight", "projector_dense2_bias",
    "broadcast_weight", "broadcast_bias",
    "gate_weight", "gate_bias",
    "output_norm_gamma", "output_norm_beta",
]


def _bc0(ap, n=128):
    """Broadcast a 1-D AP down n partitions via a stride-0 partition dim."""
    return bass.AP(tensor=ap.tensor, offset=ap.offset, ap=[[0, n]] + list(ap.ap))


def _rep0(ap, n, pos=1):
    """Insert a stride-0 free dim of extent n at position pos."""
    new = list(ap.ap)
    new.insert(pos, [0, n])
    return bass.AP(tensor=ap.tensor, offset=ap.offset, ap=new)


def build_graph(triv, debug=False):
    nc = bacc.Bacc("TRN2", target_bir_lowering=False, debug=False,
                   enable_asserts=True, num_devices=B)

    h_ext = nc.declare_dram_parameter("hidden_states", [128, ML * D], BF16,
                                      isOutput=False)
    hT_ext = nc.declare_dram_parameter("hidden_statesT", [128, ML * KD * 128], BF16,
                                       isOutput=False)
    w_ext = {}
    for n in ("aggregator_weight", "broadcast_weight", "gate_weight"):
        w_ext[n] = nc.declare_dram_parameter(n, [128, KD * D], BF16, isOutput=False)
    w_ext["projector_dense1_weight"] = nc.declare_dram_parameter(
        "projector_dense1_weight", [128, KD * HSL], BF16, isOutput=False)
    w_ext["projector_dense2_weight"] = nc.declare_dram_parameter(
        "projector_dense2_weight", [128, KHS * D], BF16, isOutput=False)
    for n in ("input_norm_gamma", "input_norm_beta", "aggregator_bias",
              "projector_norm_gamma", "projector_norm_beta",
              "projector_dense1_bias", "projector_dense2_bias",
              "broadcast_bias", "gate_bias",
              "output_norm_gamma", "output_norm_beta"):
        shape = [HSL] if n == "projector_dense1_bias" else [D]
        w_ext[n] = nc.declare_dram_parameter(n, shape, F32, isOutput=False)
    w_ext["bsel"] = nc.declare_dram_parameter("bsel", [1, B], F32, isOutput=False)
    out_ext = nc.declare_dram_parameter("out", [L, D], BF16, isOutput=True)
    dbg = {}
    if debug:
        for nm, shape in (("d_ctxrows", [B, D]), ("d_th0", [128, KD * B]),
                          ("d_tT", [128, KD * NC]), ("d_amps", [1, NC]),
                          ("d_bc", [1, D]),
                          ("d_th0half", [B, D])):
            dbg[nm] = nc.declare_dram_parameter(nm, shape, F32, isOutput=True)
        for nm, shape in (("d_gate0", [128, D]),):
            dbg[nm] = nc.declare_dram_parameter(nm, shape, BF16, isOutput=True)

    with tile.TileContext(nc) as tc:
        _build_body(nc, tc, h_ext, hT_ext, w_ext, out_ext, triv, dbg)
    nc.compile()
    return nc


def _dmajor(nc, pool, ps_pool, ident_bf, dram_ap, n, name):
    """DMA a [n*128] DRAM vector into a [128, n] d-major SBUF tile
    (tile[p, k] = v[k*128 + p]) via a bf16 [n,128] load + PE transpose."""
    rowk = pool.tile([n, 128], BF16, name="dmaj_rowk")
    nc.gpsimd.dma_start(out=rowk[:], in_=dram_ap.rearrange("(k p) -> k p", p=128))
    ps = ps_pool.tile([128, n], BF16, name="dmaj_ps")
    nc.tensor.transpose(ps[:], rowk[:], ident_bf[0:n, 0:n])
    t = pool.tile([128, n], F32, name=name)
    nc.scalar.copy(t[:], ps[:])
    return t


def _build_body(nc, tc, h_ext, hT_ext, w, out_ext, triv, dbg=None):
    dbg = dbg or {}
    import contextlib
    ctx = contextlib.ExitStack()
    with ctx:
        # ---------------- pools ----------------
        singles = ctx.enter_context(tc.tile_pool(name="singles", bufs=1))
        smalls = ctx.enter_context(tc.tile_pool(name="smalls", bufs=1))
        tstate = ctx.enter_context(tc.tile_pool(name="tstate", bufs=2))
        rows = ctx.enter_context(tc.tile_pool(name="rows", bufs=1))
        wpool = tc.alloc_tile_pool(name="wpool", bufs=1)
        dram = ctx.enter_context(tc.tile_pool(name="dram", bufs=1, space="DRAM"))

        ps_small = ctx.enter_context(tc.tile_pool(name="ps_small", bufs=2, space="PSUM"))
        ps_tr = ctx.enter_context(tc.tile_pool(name="ps_tr", bufs=1, space="PSUM"))
        ps_gate = ctx.enter_context(tc.tile_pool(name="ps_gate", bufs=3, space="PSUM"))
        ps_th = ctx.enter_context(tc.tile_pool(name="ps_th", bufs=2, space="PSUM"))

        # ---------------- collective bounce buffers ----------------
        agd_in = dram.tile([1, 8], F32)
        agd_out = dram.tile([B, 8], F32)
        ag_in = dram.tile([KD, 256], F32)
        ag_out = dram.tile([B * KD, 256], F32)
        y_in = [dram.tile([128, 256], BF16, name=f"y_in{s}")
                for s in range(2 * NUM_STEPS)]
        y_out = [dram.tile([128, 256], BF16, name=f"y_out{s}")
                 for s in range(2 * NUM_STEPS)]

        # ---------------- constants ----------------
        ident_bf = singles.tile([128, 128], BF16)
        make_identity(nc, ident_bf[:])
        ident_f32 = singles.tile([128, 128], F32)
        nc.vector.tensor_copy(ident_f32[:], ident_bf[:])
        ones_bf = singles.tile([128, 1], BF16)
        nc.vector.memset(ones_bf[:], 1.0)
        onesD_bf = singles.tile([128, 1], BF16)      # 1/1024, exact in bf16
        nc.vector.memset(onesD_bf[:], 1.0 / D)
        onesD_f32 = singles.tile([128, 1], F32)
        nc.vector.memset(onesD_f32[:], 1.0 / D)
        ones_row = singles.tile([1, 128], F32)
        nc.vector.memset(ones_row[:], 1.0)
        ones_row_bf = singles.tile([1, 128], BF16)
        nc.vector.memset(ones_row_bf[:], 1.0)
        eps1 = singles.tile([1, 1], F32)
        nc.vector.memset(eps1[:], EPS)
        eps_col = singles.tile([128, 1], F32)
        nc.vector.memset(eps_col[:], EPS)

        # dummy collective to absorb init/skew under the loads
        zdum = smalls.tile([1, 8], F32, name="zdum")
        nc.vector.memset(zdum[:], 0.0)
        nc.gpsimd.dma_start(out=agd_in[:], in_=zdum[:])
        nc.gpsimd.collective_compute(
            "AllGather", ALU.bypass, replica_groups=RG,
            ins=[agd_in.opt()], outs=[agd_out.opt()])

        # dummy collective to absorb init/skew under the loads
        zdum = smalls.tile([1, 8], F32, name="zdum")
        nc.vector.memset(zdum[:], 0.0)
        nc.gpsimd.dma_start(out=agd_in[:], in_=zdum[:])
        nc.gpsimd.collective_compute(
            "AllGather", ALU.bypass, replica_groups=RG,
            ins=[agd_in.opt()], outs=[agd_out.opt()])

        # resident (bf16) tensors
        h_bf = singles.tile([128, ML, D], BF16)       # 32KB/part
        hT_sb = wpool.tile([128, ML, KD, 128], BF16)  # 32KB/part, hT[p,m,k,l']
        gate_sb = singles.tile([128, ML, D], BF16)    # 32KB/part
        wg_bf = wpool.tile([128, KD, D], BF16)        # 16KB/part
        wagg_bf = wpool.tile([128, KD, D], BF16)      # 16KB/part
        wbc_bf = wpool.tile([128, KD, D], BF16)       # 16KB/part
        w1s_bf = wpool.tile([128, KD, HSL], BF16)     # 8KB/part
        w2s_bf = wpool.tile([128, KHS, D], BF16)      # 8KB/part

        bsel_row = smalls.tile([1, B], F32, name="bsel_row")
        nc.sync.dma_start(out=bsel_row[:], in_=w["bsel"].ap())

        # d-major vectors (only when nontrivial)
        gammaT_in = betaT_in = None
        if not triv["input_norm"]:
            gammaT_in = _dmajor(nc, singles, ps_tr, ident_bf,
                                w["input_norm_gamma"].ap(), KD, "g_in")
            betaT_in = _dmajor(nc, singles, ps_tr, ident_bf,
                               w["input_norm_beta"].ap(), KD, "b_in")
        gammaT_pr = betaT_pr = None
        if not triv["projector_norm"]:
            gammaT_pr = _dmajor(nc, singles, ps_tr, ident_bf,
                                w["projector_norm_gamma"].ap(), KD, "g_pr")
            betaT_pr = _dmajor(nc, singles, ps_tr, ident_bf,
                               w["projector_norm_beta"].ap(), KD, "b_pr")
        baggT = None
        if not triv["aggregator_bias"]:
            baggT = _dmajor(nc, singles, ps_tr, ident_bf,
                            w["aggregator_bias"].ap(), KD, "bagg")
        b1T = None
        if not triv["projector_dense1_bias"]:
            b1T = _dmajor(nc, singles, ps_tr, ident_bf,
                          w["projector_dense1_bias"].ap(), KHS, "b1")
        b2T_rep = None
        if not triv["projector_dense2_bias"]:
            b2T = _dmajor(nc, singles, ps_tr, ident_bf,
                          w["projector_dense2_bias"].ap(), KD, "b2")
            b2T_rep = _rep0(b2T[:], NC, pos=2)  # [128, KD, NC] view
        gbias_row = None
        if not triv["gate_bias"]:
            gbias_row = rows.tile([1, D], BF16, name="gbrow")
            nc.gpsimd.dma_start(out=gbias_row[:],
                                in_=w["gate_bias"].ap().rearrange("(a d) -> a d", a=1))

        # ---------------- loads (all partition-major, few big descriptors) ----
        # scalar queue: hT first (gate lhsT + context source), then W1s/W2s
        hT_src = hT_ext.ap().rearrange("p (m k l) -> p m k l", k=KD, l=128)
        for m4 in range(3):
            nc.scalar.dma_start(out=hT_sb[:, 4 * m4:4 * m4 + 4, :, :],
                                in_=hT_src[:, 4 * m4:4 * m4 + 4, :, :])
        wg_src = w["gate_weight"].ap().rearrange("p (k d) -> p k d", d=D)
        for k4 in range(KD // 4):
            nc.sync.dma_start(out=wg_bf[:, 4 * k4:4 * k4 + 4, :],
                              in_=wg_src[:, 4 * k4:4 * k4 + 4, :])
        nc.sync.dma_start(out=hT_sb[:, 12:16, :, :], in_=hT_src[:, 12:16, :, :])
        wagg_src = w["aggregator_weight"].ap().rearrange("p (k d) -> p k d", d=D)
        for k4 in range(KD // 4):
            nc.sync.dma_start(out=wagg_bf[:, 4 * k4:4 * k4 + 4, :],
                              in_=wagg_src[:, 4 * k4:4 * k4 + 4, :])
        w1_src = w["projector_dense1_weight"].ap().rearrange("p (k h) -> p k h", h=HSL)
        nc.sync.dma_start(out=w1s_bf[:], in_=w1_src)
        w2_src = w["projector_dense2_weight"].ap().rearrange("p (k d) -> p k d", d=D)
        nc.sync.dma_start(out=w2s_bf[:], in_=w2_src)

        # ---------------- gate tiles (emitted in slices to fill PE) ---------
        def emit_gate_tiles(ms):
            for m in ms:
                for n in range(2):
                    g_ps = ps_gate.tile([128, 512], F32, name="g_ps", tag="gps")
                    for k in range(KD):
                        nc.tensor.matmul(g_ps[:], hT_sb[:, m, k, :],
                                         wg_bf[:, k, n * 512:(n + 1) * 512],
                                         start=(k == 0),
                                         stop=(k == KD - 1 and gbias_row is None))
                    if gbias_row is not None:
                        nc.tensor.matmul(g_ps[:], ones_row_bf[:],
                                         gbias_row[0:1, n * 512:(n + 1) * 512],
                                         start=False, stop=True)
                    nc.scalar.activation(gate_sb[:, m, n * 512:(n + 1) * 512],
                                         g_ps[:], AF.Sigmoid)

        # ---------------- context (own batch) via DVE reduction over hT ------
        # ctx[k*128+p] = (1/L) * sum_{m,l'} hT[p, m, k, l']
        ctx_mk = smalls.tile([128, ML, KD], F32, name="ctx_mk")
        nc.vector.tensor_reduce(ctx_mk[:], hT_sb[:], axis=AX.X, op=ALU.add)
        ctxT = singles.tile([128, KD], F32)
        nc.vector.tensor_reduce(ctxT[:], ctx_mk[:].rearrange("p m k -> p k m"),
                                axis=AX.X, op=ALU.add)
        nc.vector.tensor_scalar(ctxT[:], ctxT[:], 1.0 / L, None, op0=ALU.mult)

        emit_gate_tiles(range(0, 2))

        # ---------------- own input-LN (d-major) + own thought0 row ----------
        sqc = smalls.tile([128, KD], F32, name="sqc")
        nc.vector.tensor_mul(sqc[:], ctxT[:], ctxT[:])
        cst_ps = ps_small.tile([1, 2 * KD], F32, name="cst_ps", tag="sm")
        nc.tensor.matmul(cst_ps[0:1, 0:KD], onesD_f32[:], ctxT[:],
                         start=True, stop=True)
        nc.tensor.matmul(cst_ps[0:1, KD:2 * KD], onesD_f32[:], sqc[:],
                         start=True, stop=True)
        cmr = smalls.tile([1, 2], F32, name="cmr")    # [mean, rstd]
        nc.vector.tensor_reduce(cmr[:],
                                cst_ps[:].rearrange("a (g k) -> a g k", k=KD),
                                axis=AX.X, op=ALU.add)
        cm2 = smalls.tile([1, 1], F32, name="cm2")
        nc.vector.tensor_mul(cm2[:], cmr[0:1, 0:1], cmr[0:1, 0:1])
        nc.vector.tensor_sub(cm2[:], cmr[0:1, 1:2], cm2[:])
        nc.scalar.activation(cm2[:], cm2[:], AF.Sqrt, bias=eps1[0:1, :])
        nc.vector.reciprocal(cmr[0:1, 1:2], cm2[:])
        cmr_ps = ps_small.tile([128, 2], F32, name="cmr_ps", tag="sm")
        nc.tensor.matmul(cmr_ps[:], ones_row[:], cmr[:], start=True, stop=True)
        ctxnT = smalls.tile([128, KD], F32, name="ctxnT")
        nc.vector.tensor_scalar(ctxnT[:], ctxT[:], cmr_ps[:, 0:1], cmr_ps[:, 1:2],
                                op0=ALU.subtract, op1=ALU.mult)
        if gammaT_in is not None:
            nc.vector.tensor_mul(ctxnT[:], ctxnT[:], gammaT_in[:])
            nc.vector.tensor_add(ctxnT[:], ctxnT[:], betaT_in[:])
        ctxnT_bf = smalls.tile([128, KD], BF16, name="ctxnT_bf")
        nc.vector.tensor_copy(ctxnT_bf[:], ctxnT[:])

        # own th0, d-major: th0T[p, k] = th0[k*128+p]
        th0T_ps = ps_th.tile([128, KD], F32, name="th0T_ps", tag="th")
        for dm in range(KD):
            for k in range(KD):
                nc.tensor.matmul(th0T_ps[:, dm:dm + 1],
                                 wagg_bf[:, k, dm * 128:(dm + 1) * 128],
                                 ctxnT_bf[:, k:k + 1],
                                 start=(k == 0), stop=(k == KD - 1))
        th0T = smalls.tile([128, KD], F32, name="th0T")
        if baggT is not None:
            nc.vector.tensor_add(th0T[:], th0T_ps[:], baggT[:])
        else:
            nc.scalar.copy(th0T[:], th0T_ps[:])

        # ---------------- AllGather [ctx || th0] for all batches -------------
        # single bounce write (the collective waits on ONE writer only);
        # row k on partitions 0..7 = [ctx_k(128) || th0_k(128)]
        agrow = smalls.tile([KD, 256], F32, name="agrow")
        ctxrow_ps = ps_tr.tile([KD, 128], F32, name="ctxrow_ps", tag="tr")
        nc.tensor.matmul(ctxrow_ps[:], ctxT[:], ident_f32[:], start=True, stop=True)
        nc.scalar.copy(agrow[:, 0:128], ctxrow_ps[:])
        th0r_ps = ps_tr.tile([KD, 128], F32, name="th0r_ps", tag="tr")
        nc.tensor.matmul(th0r_ps[:], th0T[:], ident_f32[:], start=True, stop=True)
        nc.scalar.copy(agrow[:, 128:256], th0r_ps[:])
        nc.sync.dma_start(out=ag_in[:], in_=agrow[:])
        nc.gpsimd.collective_compute(
            "AllGather", ALU.bypass, replica_groups=RG,
            ins=[ag_in.opt()], outs=[ag_out.opt()])
        ctxth = smalls.tile([B, KD, 2, 128], F32, name="ctxth", tag="big_scr")
        nc.sync.dma_start(out=ctxth[:],
                           in_=ag_out[:].rearrange("(r k) c -> r k c", r=B))
        if "d_ctxrows" in dbg:
            nc.sync.dma_start(out=dbg["d_ctxrows"].ap(), in_=ctxth[:, :, 0, :])
        if "d_th0half" in dbg:
            nc.sync.dma_start(out=dbg["d_th0half"].ap(), in_=ctxth[:, :, 1, :])
        # h is only needed by the tail; queue it behind the AG bounce write
        h_src = h_ext.ap().rearrange("p (m d) -> p m d", d=D)
        for m2 in range(ML // 2):
            nc.sync.dma_start(out=h_bf[:, 2 * m2:2 * m2 + 2, :],
                              in_=h_src[:, 2 * m2:2 * m2 + 2, :])

        emit_gate_tiles(range(2, 4))

        # ---------------- transpose gathered rows to d-major -----------------
        ctxr_bf = smalls.tile([B, KD, 128], BF16, name="ctxr_bf")
        nc.vector.tensor_copy(ctxr_bf[:], ctxth[:, :, 0, :])
        ctxA_bf = smalls.tile([128, KD, B], BF16, name="ctxA_bf")
        th0_all = smalls.tile([128, KD, B], F32, name="th0_all")
        for k in range(KD):
            tp = ps_tr.tile([128, B], F32, name="tp", tag="tr")
            nc.tensor.matmul(tp[:], ctxr_bf[:, k, :],
                             ident_bf[0:B, 0:B], start=True, stop=True)
            nc.scalar.copy(ctxA_bf[:, k, :], tp[:])
            tp2 = ps_tr.tile([128, B], F32, name="tp2", tag="tr")
            nc.tensor.matmul(tp2[:], ctxth[:, k, 1, :],
                             ident_f32[0:B, 0:B], start=True, stop=True)
            nc.scalar.copy(th0_all[:, k, :], tp2[:])
        if "d_th0" in dbg:
            nc.sync.dma_start(out=dbg["d_th0"].ap(),
                              in_=th0_all[:].rearrange("a k b -> a (k b)"))

        # seed 64 columns in two batch-halves (independent AR chains):
        # tT_h[:, k, b*8+p] = th0[:, k, 4h+b] * (1 + 0.02 p)
        NH = NC // 2
        BH = B // 2
        tT_h = []
        for hf in range(2):
            tTx = tstate.tile([128, KD, NH], F32, name=f"tT{hf}", tag=f"ts{hf}")
            tT_bp = tTx[:].rearrange("a k (b p) -> a k b p", p=NUM_PATHS)
            for p in range(NUM_PATHS):
                if p % 2 == 0:
                    nc.scalar.mul(tT_bp[:, :, :, p],
                                  th0_all[:, :, hf * BH:(hf + 1) * BH],
                                  1.0 + 0.02 * p)
                else:
                    nc.vector.tensor_scalar(tT_bp[:, :, :, p],
                                            th0_all[:, :, hf * BH:(hf + 1) * BH],
                                            1.0 + 0.02 * p, None, op0=ALU.mult)
            tT_h.append(tTx)

        

        # ---------------- thought steps (two pipelined half-chains) --------
        _gate_slices = {(0, 0): range(4, 6), (0, 1): range(6, 8),
                        (1, 0): range(8, 10), (1, 1): range(10, 12),
                        (2, 0): range(12, 14), (2, 1): range(14, 15),
                        (3, 0): range(15, 16)}
        for step in range(NUM_STEPS):
            for hf in range(2):
                tT = tT_h[hf]
                # LN stats over D for the 32 columns of this half
                sq = smalls.tile([128, KD, NH], BF16, name="sq", tag=f"sq{hf}")
                nc.vector.tensor_mul(sq[:], tT[:], tT[:])
                st_a = ps_small.tile([1, 256], F32, name="st_a", tag="sm")
                st_b = ps_small.tile([1, 256], F32, name="st_b", tag="sm")
                nc.tensor.matmul(st_a[:], onesD_f32[:],
                                 tT[:].rearrange("a k q -> a q k"),
                                 start=True, stop=True)
                nc.tensor.matmul(st_b[:], onesD_bf[:],
                                 sq[:].rearrange("a k q -> a q k"),
                                 start=True, stop=True)
                ms = smalls.tile([1, 2 * NH], F32, name="ms", tag=f"ms{hf}")
                nc.vector.tensor_reduce(ms[0:1, 0:NH],
                                        st_a[:].rearrange("a (q k) -> a q k", k=KD),
                                        axis=AX.X, op=ALU.add)
                nc.vector.tensor_reduce(ms[0:1, NH:2 * NH],
                                        st_b[:].rearrange("a (q k) -> a q k", k=KD),
                                        axis=AX.X, op=ALU.add)
                m2 = smalls.tile([1, NH], F32, name="m2", tag=f"m2{hf}")
                nc.vector.tensor_mul(m2[0:1, :], ms[0:1, 0:NH], ms[0:1, 0:NH])
                var = smalls.tile([1, NH], F32, name="var", tag=f"var{hf}")
                nc.vector.tensor_sub(var[0:1, :], ms[0:1, NH:2 * NH], m2[0:1, :])
                nc.scalar.activation(var[0:1, :], var[0:1, :], AF.Sqrt,
                                     bias=eps1[0:1, :])
                nc.vector.reciprocal(ms[0:1, NH:2 * NH], var[0:1, :])
                mr_ps = ps_small.tile([128, 2 * NH], F32, name="mr_ps", tag="sm")
                nc.tensor.matmul(mr_ps[:], ones_row[:], ms[:], start=True, stop=True)

                tn_bf = smalls.tile([128, KD, NH], BF16, name="tn_bf",
                                    tag=f"tn{hf}")
                tc_f = smalls.tile([128, KD, NH], F32, name="tc_f", tag=f"tc{hf}")
                nc.vector.tensor_tensor(out=tc_f[:], in0=tT[:],
                                        in1=_rep0(mr_ps[:, 0:NH], KD),
                                        op=ALU.subtract)
                if gammaT_pr is not None:
                    nc.vector.tensor_tensor(out=tc_f[:], in0=tc_f[:],
                                            in1=_rep0(mr_ps[:, NH:2 * NH], KD),
                                            op=ALU.mult)
                    nc.vector.tensor_tensor(out=tc_f[:], in0=tc_f[:],
                                            in1=_rep0(gammaT_pr[:], NH, pos=2),
                                            op=ALU.mult)
                    nc.vector.tensor_tensor(out=tn_bf[:], in0=tc_f[:],
                                            in1=_rep0(betaT_pr[:], NH, pos=2),
                                            op=ALU.add)
                else:
                    nc.vector.tensor_tensor(out=tn_bf[:], in0=tc_f[:],
                                            in1=_rep0(mr_ps[:, NH:2 * NH], KD),
                                            op=ALU.mult)

                # dense1 on my H slice: x1 [128, KHS*NH]
                x1_ps = ps_th.tile([128, KHS * NH], F32, name="x1_ps", tag="th")
                for hs in range(KHS):
                    for k in range(KD):
                        nc.tensor.matmul(x1_ps[:, hs * NH:(hs + 1) * NH],
                                         w1s_bf[:, k, hs * 128:(hs + 1) * 128],
                                         tn_bf[:, k, :],
                                         start=(k == 0), stop=(k == KD - 1))
                # tanh-gelu, reading dense1 psum directly
                xs = smalls.tile([128, KHS * NH], F32, name="gelu_x",
                                 tag=f"xs{hf}")
                if b1T is not None:
                    for hs in range(KHS):
                        nc.scalar.activation(xs[:, hs * NH:(hs + 1) * NH],
                                             x1_ps[:, hs * NH:(hs + 1) * NH],
                                             AF.Identity, bias=b1T[:, hs:hs + 1])
                else:
                    nc.vector.tensor_copy(xs[:], x1_ps[:])
                xin = xs[:]
                u = smalls.tile([128, KHS * NH], F32, name="gelu_u", tag=f"u{hf}")
                nc.vector.tensor_mul(u[:], xin, xin)
                nc.vector.tensor_mul(u[:], u[:], xin)
                nc.vector.scalar_tensor_tensor(u[:], u[:], 0.044715, xin,
                                               op0=ALU.mult, op1=ALU.add)
                nc.scalar.activation(u[:], u[:], AF.Tanh, scale=0.7978845608028654)
                nc.vector.scalar_tensor_tensor(u[:], u[:], 1.0, xin,
                                               op0=ALU.add, op1=ALU.mult)
                x1_bf = smalls.tile([128, KHS, NH], BF16, name="x1_bf",
                                    tag=f"x1{hf}")
                nc.vector.tensor_scalar(x1_bf[:].rearrange("a b c -> a (b c)"),
                                        u[:], 0.5, None, op0=ALU.mult)

                # dense2 partial: y [128, KD*NH]
                y_ps = ps_th.tile([128, KD * NH], F32, name="y_ps", tag="th")
                for dm in range(KD):
                    for hk in range(KHS):
                        nc.tensor.matmul(y_ps[:, dm * NH:(dm + 1) * NH],
                                         w2s_bf[:, hk, dm * 128:(dm + 1) * 128],
                                         x1_bf[:, hk, :],
                                         start=(hk == 0), stop=(hk == KHS - 1))
                y_sb = smalls.tile([128, KD * NH], BF16, name="y_sb",
                                   tag=f"ysb{hf}")
                nc.scalar.copy(y_sb[:], y_ps[:])
                sh = 2 * step + hf
                nc.sync.dma_start(out=y_in[sh][:], in_=y_sb[:])
                nc.gpsimd.collective_compute(
                    "AllReduce", ALU.add, replica_groups=RG,
                    ins=[y_in[sh].opt()], outs=[y_out[sh].opt()])
                if step == 0 and hf == 0:
                    # Wbc load late so it never delays the hot-path loads
                    wbc_src = w["broadcast_weight"].ap().rearrange(
                        "p (k d) -> p k d", d=D)
                    nc.sync.dma_start(out=wbc_bf[:], in_=wbc_src)
                if (step, hf) in _gate_slices:
                    emit_gate_tiles(_gate_slices[(step, hf)])
                y_rd = smalls.tile([128, KD * NH], BF16, name="y_rd",
                                   tag=f"yrd{hf}")
                nc.sync.dma_start(out=y_rd[:], in_=y_out[sh][:])

                tT_new = tstate.tile([128, KD, NH], F32, name="tT_new",
                                     tag=f"ts{hf}")
                yv = y_rd[:].rearrange("a (k q) -> a k q", k=KD)
                if b2T_rep is not None:
                    b2r = _rep0(b2T[:], NH, pos=2)
                    nc.vector.tensor_add(tT_new[:], yv, b2r)
                    nc.vector.tensor_add(tT_new[:], tT_new[:], tT[:])
                else:
                    nc.vector.tensor_add(tT_new[:], yv, tT[:])
                tT_h[hf] = tT_new

        # merge halves for scores/collapse
        tT_full = smalls.tile([128, KD, NC], F32, name="tT_full")
        nc.vector.tensor_copy(tT_full[:, :, 0:NH], tT_h[0][:])
        nc.vector.tensor_copy(tT_full[:, :, NH:NC], tT_h[1][:])
        tT = tT_full
        tT_bf = smalls.tile([128, KD, NC], BF16, name="tT_bf")
        nc.vector.tensor_copy(tT_bf[:], tT_full[:])
        # scores: elementwise tT*ctx (ctx per batch), partition-sum via one
        # ones-matmul, then k-reduce
        prodc = smalls.tile([128, KD, NC], BF16, name="prodc")
        nc.vector.tensor_tensor(
            out=prodc[:].rearrange("a k (b p) -> a k b p", p=NUM_PATHS),
            in0=tT_bf[:].rearrange("a k (b p) -> a k b p", p=NUM_PATHS),
            in1=_rep0(ctxA_bf[:], NUM_PATHS, pos=3), op=ALU.mult)
        sc1_ps = ps_small.tile([1, 512], F32, name="sc1_ps", tag="sm")
        nc.tensor.matmul(sc1_ps[:], ones_bf[:],
                         prodc[:].rearrange("a k q -> a (k q)"),
                         start=True, stop=True)
        sc = smalls.tile([1, NC], F32, name="sc")
        nc.vector.tensor_reduce(sc[:],
                                sc1_ps[:].rearrange("a (k q) -> a q k", q=NC),
                                axis=AX.X, op=ALU.add)
        nc.vector.tensor_scalar(sc[:], sc[:], INV_SQRT_D, None, op0=ALU.mult)
        scv = sc[:].rearrange("a (b p) -> a b p", p=NUM_PATHS)
        negmax = smalls.tile([1, B], F32, name="negmax")
        nc.vector.tensor_reduce(negmax[:], scv, axis=AX.X, op=ALU.max,
                                negate=True)
        exv = smalls.tile([1, NC], F32, name="exv")
        nc.vector.tensor_tensor(
            out=exv[:].rearrange("a (b p) -> a b p", p=NUM_PATHS),
            in0=scv, in1=_rep0(negmax[:], NUM_PATHS, pos=2), op=ALU.add)
        nc.scalar.activation(exv[:], exv[:], AF.Exp)
        esum = smalls.tile([1, B], F32, name="esum")
        nc.vector.tensor_reduce(
            esum[:], exv[:].rearrange("a (b p) -> a b p", p=NUM_PATHS),
            axis=AX.X, op=ALU.add)
        rsum = smalls.tile([1, B], F32, name="rsum")
        nc.vector.reciprocal(rsum[:], esum[:])
        amps0 = smalls.tile([1, NC], F32, name="amps0")
        nc.vector.tensor_tensor(
            out=amps0[:].rearrange("a (b p) -> a b p", p=NUM_PATHS),
            in0=exv[:].rearrange("a (b p) -> a b p", p=NUM_PATHS),
            in1=_rep0(rsum[:], NUM_PATHS, pos=2), op=ALU.mult)
        mask = smalls.tile([1, NC], F32, name="mask")
        nc.vector.tensor_scalar(mask[:], amps0[:], PRUNE, None, op0=ALU.is_ge)
        pruned = smalls.tile([1, NC], F32, name="pruned")
        nc.vector.tensor_mul(pruned[:], amps0[:], mask[:])
        psum_s = smalls.tile([1, B], F32, name="psum_s")
        nc.vector.tensor_reduce(
            psum_s[:], pruned[:].rearrange("a (b p) -> a b p", p=NUM_PATHS),
            axis=AX.X, op=ALU.add)
        nc.vector.tensor_scalar(psum_s[:], psum_s[:], EPS, None, op0=ALU.add)
        rr = smalls.tile([1, B], F32, name="rr")
        nc.vector.reciprocal(rr[:], psum_s[:])
        ampsF = smalls.tile([1, NC], F32, name="ampsF")
        nc.vector.tensor_tensor(
            out=ampsF[:].rearrange("a (b p) -> a b p", p=NUM_PATHS),
            in0=pruned[:].rearrange("a (b p) -> a b p", p=NUM_PATHS),
            in1=_rep0(rr[:], NUM_PATHS, pos=2), op=ALU.mult)

        if "d_tT" in dbg:
            nc.sync.dma_start(out=dbg["d_tT"].ap(),
                              in_=tT[:].rearrange("a k q -> a (k q)"))
        if "d_amps" in dbg:
            nc.sync.dma_start(out=dbg["d_amps"].ap(), in_=ampsF[:])

        # ---------------- collapse + own-batch bc ----------------
        ab_ps = ps_small.tile([128, NC], F32, name="ab_ps", tag="sm")
        nc.tensor.matmul(ab_ps[:], ones_row[0:1, :], ampsF[:], start=True, stop=True)
        amps_sb = smalls.tile([128, NC], F32, name="amps_sb")
        nc.scalar.copy(amps_sb[:], ab_ps[:])
        prod = smalls.tile([128, KD, NC], F32, name="prod", tag="big_scr")
        nc.vector.tensor_tensor(out=prod[:], in0=tT[:], in1=_rep0(amps_sb[:], KD),
                                op=ALU.mult)
        finalT = smalls.tile([128, KD, B], F32, name="finalT")
        nc.vector.tensor_reduce(
            finalT[:], prod[:].rearrange("a k (b p) -> a k b p", p=NUM_PATHS),
            axis=AX.X, op=ALU.add)
        # select own batch column via the bsel one-hot row (physical bcast)
        bsel_ps = ps_small.tile([128, B], F32, name="bsel_ps", tag="sm")
        nc.tensor.matmul(bsel_ps[:], ones_row[:], bsel_row[:], start=True, stop=True)
        fsel = smalls.tile([128, KD, B], F32, name="fsel", tag="big_scr")
        nc.vector.tensor_tensor(out=fsel[:], in0=finalT[:],
                                in1=_rep0(bsel_ps[:], KD), op=ALU.mult)
        fown = smalls.tile([128, KD], F32, name="fown")
        nc.vector.tensor_reduce(fown[:], fsel[:], axis=AX.X, op=ALU.add)
        fown_bf = smalls.tile([128, KD], BF16, name="fown_bf")
        nc.vector.tensor_copy(fown_bf[:], fown[:])

        # bc row [1, D] = fown @ Wbc, then broadcast down 128 partitions
        bb_row = None
        if not triv["broadcast_bias"]:
            bb_row = rows.tile([1, D], F32, name="bbrow")
            nc.sync.dma_start(out=bb_row[:],
                              in_=w["broadcast_bias"].ap().rearrange("(a d) -> a d", a=1))
        bc_bf = singles.tile([128, D], BF16)
        bc_row = smalls.tile([1, D], F32, name="bc_row")
        for n in range(2):
            bc_ps = ps_small.tile([1, 512], F32, name="bc_ps", tag="sm")
            for k in range(KD):
                nc.tensor.matmul(bc_ps[:], fown_bf[:, k:k + 1],
                                 wbc_bf[:, k, n * 512:(n + 1) * 512],
                                 start=(k == 0), stop=(k == KD - 1))
            if bb_row is not None:
                nc.vector.tensor_add(bc_row[0:1, n * 512:(n + 1) * 512], bc_ps[:],
                                     bb_row[0:1, n * 512:(n + 1) * 512])
            else:
                nc.scalar.copy(bc_row[0:1, n * 512:(n + 1) * 512], bc_ps[:])
            bcb_ps = ps_gate.tile([128, 512], F32, name="bcb_ps", tag="gps")
            nc.tensor.matmul(bcb_ps[:], ones_row[0:1, :],
                             bc_row[0:1, n * 512:(n + 1) * 512],
                             start=True, stop=True)
            nc.scalar.copy(bc_bf[:, n * 512:(n + 1) * 512], bcb_ps[:])
        if "d_bc" in dbg:
            nc.sync.dma_start(out=dbg["d_bc"].ap(), in_=bc_row[:])

        # release weights; final-phase pools reuse the space
        wpool.release()
        fin = ctx.enter_context(tc.tile_pool(name="fin", bufs=3))
        fin1 = ctx.enter_context(tc.tile_pool(name="fin1", bufs=1))
        gamma_out_b = beta_out_b = None
        if not triv["output_norm"]:
            gamma_out_b = fin1.tile([128, D], F32)
            nc.sync.dma_start(out=gamma_out_b[:], in_=_bc0(w["output_norm_gamma"].ap()))
            beta_out_b = fin1.tile([128, D], F32)
            nc.sync.dma_start(out=beta_out_b[:], in_=_bc0(w["output_norm_beta"].ap()))

        # ---------------- final LN + output (bf16) ----------------
        # engine-batched passes; elementwise work split DVE/gpsimd, writes
        # split scalar/sync so no single queue bounds the tail
        if "d_gate0" in dbg:
            nc.sync.dma_start(out=dbg["d_gate0"].ap(), in_=gate_sb[:, 0, :])
        pre_all = fin1.tile([128, ML, D], BF16)      # 32KB/part
        rs_all = fin1.tile([128, ML, 2], F32)        # [rowsum, rowsumsq] per m
        sq_scr = fin1.tile([128, D], BF16)           # accum side-effect scratch
        # pass A: pre = h + gate*bc (p1 on gpsimd, pre-add on DVE, sq on ACT)
        for m in range(ML):
            p1 = fin.tile([128, D], BF16, name="p1")
            nc.vector.tensor_mul(p1[:], gate_sb[:, m, :], bc_bf[:])
            nc.vector.scalar_tensor_tensor(pre_all[:, m, :], p1[:], 1.0,
                                           h_bf[:, m, :],
                                           op0=ALU.mult, op1=ALU.add,
                                           accum_out=rs_all[:, m, 0:1])
            nc.scalar.activation(sq_scr[:], pre_all[:, m, :], AF.Square,
                                 accum_out=rs_all[:, m, 1:2])
        # pass B: per-row mean/rstd for all tiles at once
        mv_all = fin1.tile([128, ML, 2], F32)
        nc.vector.tensor_scalar(mv_all[:], rs_all[:], 1.0 / D, None, op0=ALU.mult)
        var_all = fin1.tile([128, ML], F32)
        nc.vector.tensor_tensor(out=var_all[:], in0=mv_all[:, :, 0],
                                in1=mv_all[:, :, 0], op=ALU.mult)
        nc.vector.tensor_sub(var_all[:], mv_all[:, :, 1], var_all[:])
        sd_all = fin1.tile([128, ML], F32)
        nc.scalar.activation(sd_all[:], var_all[:], AF.Sqrt, bias=eps_col[:])
        rstd_all = fin1.tile([128, ML], F32)
        nc.vector.reciprocal(rstd_all[:], sd_all[:])
        negmr_all = fin1.tile([128, ML], F32)
        nc.vector.tensor_tensor(out=negmr_all[:], in0=mv_all[:, :, 0],
                                in1=rstd_all[:], op=ALU.mult)
        nc.vector.tensor_scalar(negmr_all[:], negmr_all[:], -1.0, None,
                                op0=ALU.mult)
        # pass C: normalize + write out (alternate DVE/gpsimd, scalar/sync)
        for m in range(ML):
            o = fin.tile([128, D], BF16, name="o")
            if m % 4 == 0:
                nc.vector.tensor_scalar(o[:], pre_all[:, m, :], mv_all[:, m, 0:1],
                                        rstd_all[:, m:m + 1],
                                        op0=ALU.subtract, op1=ALU.mult)
            else:
                nc.scalar.activation(o[:], pre_all[:, m, :], AF.Identity,
                                     bias=negmr_all[:, m:m + 1],
                                     scale=rstd_all[:, m:m + 1])
            if gamma_out_b is not None:
                nc.vector.tensor_mul(o[:], o[:], gamma_out_b[:])
                nc.vector.tensor_add(o[:], o[:], beta_out_b[:])
            q = nc.scalar if m % 2 == 0 else nc.sync
            q.dma_start(out=out_ext.ap()[m * 128:(m + 1) * 128, :], in_=o[:])


def _triv_flags(inputs):
    def ones(x):
        return bool(np.all(np.asarray(x) == 1.0))

    def zeros(x):
        return bool(np.all(np.asarray(x) == 0.0))

    return {
        "input_norm": ones(inputs["input_norm_gamma"]) and zeros(inputs["input_norm_beta"]),
        "projector_norm": ones(inputs["projector_norm_gamma"]) and zeros(inputs["projector_norm_beta"]),
        "output_norm": ones(inputs["output_norm_gamma"]) and zeros(inputs["output_norm_beta"]),
        "aggregator_bias": zeros(inputs["aggregator_bias"]),
        "projector_dense1_bias": zeros(inputs["projector_dense1_bias"]),
        "projector_dense2_bias": zeros(inputs["projector_dense2_bias"]),
        "broadcast_bias": zeros(inputs["broadcast_bias"]),
        "gate_bias": zeros(inputs["gate_bias"]),
    }


_GRAPH_CACHE = {}

BF16_INPUTS = ("hidden_states", "aggregator_weight", "projector_dense1_weight",
               "projector_dense2_weight", "broadcast_weight", "gate_weight")


def prep_in_maps(inputs):
    """Build per-core input maps: core r gets batch r of hidden_states plus
    its H-slice of W1/W2 and a one-hot batch selector; other weights are
    replicated.  Big tensors are host-cast to bf16 and repacked
    partition-major."""
    import ml_dtypes

    def pmajor(a):
        """[K*128, N] -> [128, K*N] partition-major contiguous repack."""
        k = a.shape[0] // 128
        return np.ascontiguousarray(
            a.reshape(k, 128, -1).transpose(1, 0, 2).reshape(128, -1))

    hs = np.ascontiguousarray(
        np.asarray(inputs["hidden_states"], dtype=np.float32).astype(ml_dtypes.bfloat16))
    assert hs.shape == (B, L, D)
    com = {}
    for n in WEIGHT_NAMES:
        a = np.asarray(inputs[n], dtype=np.float32)
        if n in BF16_INPUTS:
            a = a.astype(ml_dtypes.bfloat16)
        com[n] = np.ascontiguousarray(a)
    for n in ("aggregator_weight", "broadcast_weight", "gate_weight"):
        com[n] = pmajor(com[n])
    in_maps = []
    for r in range(B):
        m = dict(com)
        m["hidden_states"] = pmajor(hs[r])
        # hT[p, m, k, l'] = h[m*128+l', k*128+p]
        m["hidden_statesT"] = np.ascontiguousarray(
            hs[r].T.reshape(KD, 128, ML, 128).transpose(1, 2, 0, 3).reshape(128, -1))
        m["projector_dense1_weight"] = pmajor(
            com["projector_dense1_weight"][:, r * HSL:(r + 1) * HSL])
        m["projector_dense2_weight"] = pmajor(
            com["projector_dense2_weight"][r * HSL:(r + 1) * HSL, :])
        m["projector_dense1_bias"] = np.ascontiguousarray(
            com["projector_dense1_bias"][r * HSL:(r + 1) * HSL])
        sel = np.zeros((1, B), dtype=np.float32)
        sel[0, r] = 1.0
        m["bsel"] = sel
        in_maps.append(m)
    return in_maps


def kernel(**inputs):
    triv = _triv_flags(inputs)
    key = tuple(sorted(triv.items()))
    if key not in _GRAPH_CACHE:
        _GRAPH_CACHE[key] = build_graph(triv)
    nc = _GRAPH_CACHE[key]
    in_maps = prep_in_maps(inputs)
    res = run_bass_kernel_spmd(nc, in_maps, core_ids=list(range(B)))
    out = np.stack([np.asarray(res.results[b]["out"]).astype(np.float32)
                    for b in range(B)], axis=0)
    return out
